# revision 1
# baseline (speedup 1.0000x reference)
import os
import sys
from contextlib import ExitStack

import numpy as np

sys.path.insert(0, "/opt/trn_rl_repo")

import concourse.bass as bass
from concourse import bacc
import concourse.tile as tile
from concourse import mybir
from concourse.bass_utils import run_bass_kernel_spmd

# Problem constants (hardcoded per contract)
B, T, N, F_IN, F_OUT = 64, 12, 325, 32, 128
NC = 8          # cores
BL = B // NC    # batch per core = 8
NP = 384        # padded node count (3 x 128)
NJ = 3          # node chunks
CX = F_IN + 1   # x channels + ones channel (bias trick) = 33
CH = F_OUT      # 128
NOPS = 5        # I, A_out, A_in, A_out2, A_in2
F32 = mybir.dt.float32

_CACHE = {}


def _build_bass():
    nc = bacc.Bacc(None, target_bir_lowering=False)
    x_d = nc.dram_tensor("xin", [128, NJ, T, BL, CX], F32, kind="ExternalInput")
    a_d = nc.dram_tensor("amat", [128, NJ, NOPS, NP], F32, kind="ExternalInput")
    wzrx_d = nc.dram_tensor("wzrx", [CX, NOPS, 2 * F_OUT], F32, kind="ExternalInput")
    wzrh_d = nc.dram_tensor("wzrh", [CH, NOPS, 2 * F_OUT], F32, kind="ExternalInput")
    whx_d = nc.dram_tensor("whx", [CX, NOPS, F_OUT], F32, kind="ExternalInput")
    whh_d = nc.dram_tensor("whh", [CH, NOPS, F_OUT], F32, kind="ExternalInput")
    y_d = nc.dram_tensor("y", [128, T, NJ, BL, F_OUT], F32, kind="ExternalOutput")

    with tile.TileContext(nc) as tc, ExitStack() as ctx:
        const = ctx.enter_context(tc.tile_pool(name="const", bufs=1))
        state = ctx.enter_context(tc.tile_pool(name="state", bufs=1))
        gpool = ctx.enter_context(tc.tile_pool(name="g", bufs=3))
        gcp = ctx.enter_context(tc.tile_pool(name="gcp", bufs=2))
        mid = ctx.enter_context(tc.tile_pool(name="mid", bufs=2))
        spool = ctx.enter_context(tc.tile_pool(name="s", bufs=3))
        psg = ctx.enter_context(tc.tile_pool(name="psg", bufs=2, space="PSUM"))
        psx = ctx.enter_context(tc.tile_pool(name="psx", bufs=2, space="PSUM"))
        psz = ctx.enter_context(tc.tile_pool(name="psz", bufs=2, space="PSUM"))
        psc = ctx.enter_context(tc.tile_pool(name="psc", bufs=2, space="PSUM"))

        xin = const.tile([128, NJ, T, BL, CX], F32)
        amat = const.tile([128, NJ, NOPS, NP], F32)
        wx = const.tile([CX, NOPS, 3 * F_OUT], F32)
        wh = const.tile([CH, NOPS, 3 * F_OUT], F32)
        nc.sync.dma_start(xin[:], x_d[:])
        nc.sync.dma_start(amat[:], a_d[:])
        nc.sync.dma_start(wx[:, :, 0:2 * F_OUT], wzrx_d[:])
        nc.sync.dma_start(wh[:, :, 0:2 * F_OUT], wzrh_d[:])
        nc.sync.dma_start(wx[:, :, 2 * F_OUT:], whx_d[:])
        nc.sync.dma_start(wh[:, :, 2 * F_OUT:], whh_d[:])

        hs = state.tile([128, NJ, BL, CH], F32)  # node-major hidden state
        nc.gpsimd.memset(hs[:], 0.0)

        def graph_ops(lhs_fn, cpart, gtile, ps_pool):
            # gtile[c, P, d] = sum_s lhs[s, c] * amat[s, P, d]  (channel-major result)
            for P in range(NOPS):
                ps = ps_pool.tile([cpart, NP], F32)
                for j in range(NJ):
                    nc.tensor.matmul(ps[:], lhs_fn(j), amat[:, j, P, :],
                                     start=(j == 0), stop=(j == NJ - 1))
                nc.scalar.copy(gtile[:, P, :], ps[:])

        FO = F_OUT

        def zr_graph(t, b, gs):
            gx = gpool.tile([CX, NOPS, NP], F32, tag="gx")
            graph_ops(lambda j: xin[:, j, t, b, :], CX, gx, psx)
            gh = gpool.tile([CH, NOPS, NP], F32, tag="gh")
            graph_ops(lambda j: hs[:, j, b, :], CH, gh, psg)
            gs[b] = (gx, gh)

        def zr_gates(b, gs, zs):
            gx, gh = gs[b]
            zt = mid.tile([128, NJ, FO], F32, tag="zt")
            hr = mid.tile([128, NJ, CH], F32, tag="hr")
            for m in range(NJ):
                pz = psz.tile([128, 2 * FO], F32)
                for P in range(NOPS):
                    nc.tensor.matmul(pz[:], gx[:, P, bass.ts(m, 128)],
                                     wx[:, P, 0:2 * FO], start=(P == 0), stop=False)
                for P in range(NOPS):
                    nc.tensor.matmul(pz[:], gh[:, P, bass.ts(m, 128)],
                                     wh[:, P, 0:2 * FO], start=False,
                                     stop=(P == NOPS - 1))
                nc.scalar.activation(zt[:, m, :], pz[:, 0:FO],
                                     mybir.ActivationFunctionType.Sigmoid)
                rt = spool.tile([128, FO], F32, tag="tmp")
                nc.scalar.activation(rt[:], pz[:, FO:2 * FO],
                                     mybir.ActivationFunctionType.Sigmoid)
                nc.vector.tensor_mul(hr[:, m, :], hs[:, m, b, :], rt[:])
            zs[b] = (zt, hr)

        def cand_graph(b, zs, cs):
            zt, hr = zs[b]
            gc = gcp.tile([CH, NOPS, NP], F32, tag="gc")
            graph_ops(lambda j: hr[:, j, :], CH, gc, psg)
            cs[b] = gc

        def cand_gates(t, b, gs, zs, cs):
            gx, _ = gs[b]
            zt, _ = zs[b]
            gc = cs[b]
            for m in range(NJ):
                pc = psc.tile([128, FO], F32)
                for P in range(NOPS):
                    nc.tensor.matmul(pc[:], gx[:, P, bass.ts(m, 128)],
                                     wx[:, P, 2 * FO:], start=(P == 0), stop=False)
                for P in range(NOPS):
                    nc.tensor.matmul(pc[:], gc[:, P, bass.ts(m, 128)],
                                     wh[:, P, 2 * FO:], start=False,
                                     stop=(P == NOPS - 1))
                ht = spool.tile([128, FO], F32, tag="tmp")
                nc.scalar.activation(ht[:], pc[:],
                                     mybir.ActivationFunctionType.Tanh)
                d1 = spool.tile([128, FO], F32, tag="tmp")
                nc.vector.tensor_sub(d1[:], hs[:, m, b, :], ht[:])
                d2 = spool.tile([128, FO], F32, tag="tmp")
                nc.vector.tensor_mul(d2[:], zt[:, m, :], d1[:])
                nc.vector.tensor_add(hs[:, m, b, :], ht[:], d2[:])
            nc.sync.dma_start(y_d[:, t, :, b, :], hs[:, :, b, :])

        for t in range(T):
            gs, zs, cs = {}, {}, {}
            zr_graph(t, 0, gs)
            zr_graph(t, 1, gs)
            zr_gates(0, gs, zs)
            for b in range(BL):
                if b + 2 < BL:
                    zr_graph(t, b + 2, gs)
                cand_graph(b, zs, cs)
                if b + 1 < BL:
                    zr_gates(b + 1, gs, zs)
                cand_gates(t, b, gs, zs, cs)
    nc.compile()
    return nc


def _prep_consts(edge_index, edge_weight, Wz, bz, Wr, br, Wh, bh):
    row = edge_index[0].astype(np.int64)
    col = edge_index[1].astype(np.int64)
    w = edge_weight.astype(np.float32)
    deg_out = np.zeros(N, np.float32)
    deg_in = np.zeros(N, np.float32)
    np.add.at(deg_out, row, w)
    np.add.at(deg_in, col, w)
    norm_out = (1.0 / deg_out)[row]
    norm_in = (1.0 / deg_in)[row]  # quirk: indexed by row
    perm = np.argsort(col * N + row, kind="stable")
    A_out = np.zeros((N, N), np.float32)
    A_in = np.zeros((N, N), np.float32)
    np.add.at(A_out, (col, row), norm_out)
    np.add.at(A_in, (row[perm], col[perm]), norm_in)  # norm_in unpermuted
    I = np.eye(N, dtype=np.float32)
    A_out2 = 2.0 * (A_out @ A_out) - I
    A_in2 = 2.0 * (A_in @ A_in) - I

    amat = np.zeros((NOPS, NP, NP), np.float32)  # [P, d, s]
    for i, A in enumerate([I, A_out, A_in, A_out2, A_in2]):
        amat[i, :N, :N] = A
    # rhs layout [s%128, j, P, d]: AT[P][s, d] = A[d, s]
    amat_r = amat.transpose(2, 0, 1).reshape(NJ, 128, NOPS, NP).transpose(1, 0, 2, 3)
    amat_r = np.ascontiguousarray(amat_r)

    def terms(W):  # W: [2, 3, C, co] -> list of 5 [C, co]
        return [W[0, 0] + W[1, 0], W[0, 1], W[1, 1], W[0, 2], W[1, 2]]

    tz, tr, th = terms(Wz), terms(Wr), terms(Wh)
    wzrx = np.zeros((CX, NOPS, 2 * F_OUT), np.float32)
    wzrh = np.zeros((CH, NOPS, 2 * F_OUT), np.float32)
    whx = np.zeros((CX, NOPS, F_OUT), np.float32)
    whh = np.zeros((CH, NOPS, F_OUT), np.float32)
    for P in range(NOPS):
        wzr = np.concatenate([tz[P], tr[P]], axis=1)  # [C, 256]
        wzrx[:F_IN, P] = wzr[:F_IN]
        wzrh[:, P] = wzr[F_IN:]
        whx[:F_IN, P] = th[P][:F_IN]
        whh[:, P] = th[P][F_IN:]
    wzrx[F_IN, 0] = np.concatenate([bz, br])  # bias via ones channel, op I only
    whx[F_IN, 0] = bh
    return amat_r, wzrx, wzrh, whx, whh


def kernel(X, edge_index, edge_weight, Wz, bz, Wr, br, Wh, bh):
    X = np.asarray(X, np.float32)
    amat_r, wzrx, wzrh, whx, whh = _prep_consts(
        np.asarray(edge_index), np.asarray(edge_weight, np.float32),
        np.asarray(Wz, np.float32), np.asarray(bz, np.float32),
        np.asarray(Wr, np.float32), np.asarray(br, np.float32),
        np.asarray(Wh, np.float32), np.asarray(bh, np.float32))

    if "nc" not in _CACHE:
        _CACHE["nc"] = _build_bass()
    nc = _CACHE["nc"]

    in_maps = []
    for c in range(NC):
        Xl = X[c * BL:(c + 1) * BL]  # [BL, T, N, F_IN]
        Xp = np.zeros((BL, T, NP, CX), np.float32)
        Xp[:, :, :N, :F_IN] = Xl
        Xp[:, :, :, F_IN] = 1.0
        # -> [p, j, t, b, c]
        Xp = Xp.reshape(BL, T, NJ, 128, CX).transpose(3, 2, 1, 0, 4)
        in_maps.append({
            "xin": np.ascontiguousarray(Xp),
            "amat": amat_r, "wzrx": wzrx, "wzrh": wzrh,
            "whx": whx, "whh": whh,
        })

    trace = bool(int(os.environ.get("KERNEL_TRACE", "0")))
    res = run_bass_kernel_spmd(nc, in_maps, core_ids=list(range(NC)), trace=trace)
    _CACHE["last_result"] = res

    out = np.empty((B, T, N, F_OUT), np.float32)
    for c in range(NC):
        y = res.results[c]["y"]  # [128, T, NJ, BL, F_OUT]
        y = y.reshape(128, T, NJ, BL, F_OUT).transpose(3, 1, 2, 0, 4)
        out[c * BL:(c + 1) * BL] = y.reshape(BL, T, NP, F_OUT)[:, :, :N, :]
    return out



# revision 17
# speedup vs baseline: 3.8441x; 3.8441x over previous
import os
import sys
from contextlib import ExitStack

import ml_dtypes
import numpy as np

sys.path.insert(0, "/opt/trn_rl_repo")

import concourse.bass as bass
from concourse import bacc
import concourse.tile as tile
from concourse import mybir
from concourse.bass_utils import run_bass_kernel_spmd

# Problem constants (hardcoded per contract)
B, T, N, F_IN, F_OUT = 64, 12, 325, 32, 128
NC = 8          # cores
BL = B // NC    # batch per core = 8
NP = 384        # padded node count (3 x 128)
NJ = 3          # node chunks
CX = F_IN + 1   # x channels + ones channel (bias trick) = 33
CH = F_OUT      # 128
NOPS = 5        # I, A_out, A_in, A_out2, A_in2
F32 = mybir.dt.float32
F32R = mybir.dt.float32r
BF16 = mybir.dt.bfloat16

_CACHE = {}


def _build_bass():
    nc = bacc.Bacc(None, target_bir_lowering=False)
    x_d = nc.dram_tensor("xin", [128, NJ, T, BL, CX], BF16, kind="ExternalInput")
    a_d = nc.dram_tensor("amat", [128, NJ, NOPS, NP], BF16, kind="ExternalInput")
    wzrx_d = nc.dram_tensor("wzrx", [CX, NOPS, 2 * F_OUT], BF16, kind="ExternalInput")
    wzrh_d = nc.dram_tensor("wzrh", [CH, NOPS, 2 * F_OUT], BF16, kind="ExternalInput")
    whx_d = nc.dram_tensor("whx", [CX, NOPS, F_OUT], BF16, kind="ExternalInput")
    whh_d = nc.dram_tensor("whh", [CH, NOPS, F_OUT], BF16, kind="ExternalInput")
    y_d = nc.dram_tensor("y", [128, T, NJ, BL, F_OUT], F32, kind="ExternalOutput")

    with tile.TileContext(nc) as tc, ExitStack() as ctx:
        const = ctx.enter_context(tc.tile_pool(name="const", bufs=1))
        state = ctx.enter_context(tc.tile_pool(name="state", bufs=1))
        gpool = ctx.enter_context(tc.tile_pool(name="g", bufs=3))
        gcp = ctx.enter_context(tc.tile_pool(name="gcp", bufs=2))
        mid = ctx.enter_context(tc.tile_pool(name="mid", bufs=2))
        spool = ctx.enter_context(tc.tile_pool(name="s", bufs=3))
        psg = ctx.enter_context(tc.tile_pool(name="psg", bufs=2, space="PSUM"))
        psx = ctx.enter_context(tc.tile_pool(name="psx", bufs=2, space="PSUM"))
        psz = ctx.enter_context(tc.tile_pool(name="psz", bufs=2, space="PSUM"))
        psc = ctx.enter_context(tc.tile_pool(name="psc", bufs=2, space="PSUM"))

        xin = const.tile([128, NJ, T, BL, CX], BF16)
        amat = const.tile([128, NJ, NOPS, NP], BF16)
        wx = const.tile([CX, NOPS, 3 * F_OUT], BF16)
        wh = const.tile([CH, NOPS, 3 * F_OUT], BF16)
        nc.sync.dma_start(xin[:], x_d[:])
        nc.sync.dma_start(amat[:], a_d[:])
        nc.sync.dma_start(wx[:, :, 0:2 * F_OUT], wzrx_d[:])
        nc.sync.dma_start(wh[:, :, 0:2 * F_OUT], wzrh_d[:])
        nc.sync.dma_start(wx[:, :, 2 * F_OUT:], whx_d[:])
        nc.sync.dma_start(wh[:, :, 2 * F_OUT:], whh_d[:])

        hs = state.tile([128, NJ, BL, CH], F32)  # node-major hidden state
        hsb = state.tile([128, NJ, BL, CH], BF16)  # bf16 copy for matmul lhsT
        nc.gpsimd.memset(hs[:], 0.0)
        nc.gpsimd.memset(hsb[:], 0.0)

        def graph_ops(lhs_fn, cpart, gtile, ps_pool):
            # gtile[c, P, d] = sum_s lhs[s, c] * amat[s, P, d]  (channel-major result)
            for P in range(NOPS):
                ps = ps_pool.tile([cpart, NP], F32)
                for j in range(NJ):
                    nc.tensor.matmul(ps[:], lhs_fn(j), amat[:, j, P, :],
                                     start=(j == 0), stop=(j == NJ - 1))
                nc.scalar.copy(gtile[:, P, :], ps[:])

        FO = F_OUT

        def zr_graph(t, b, gs):
            gx = gpool.tile([CX, NOPS, NP], BF16, tag="gx")
            graph_ops(lambda j: xin[:, j, t, b, :], CX, gx, psx)
            gh = gpool.tile([CH, NOPS, NP], BF16, tag="gh")
            graph_ops(lambda j: hsb[:, j, b, :], CH, gh, psg)
            gs[b] = (gx, gh)

        def zr_gates(b, gs, zs):
            gx, gh = gs[b]
            zt = mid.tile([128, NJ, FO], F32, tag="zt")
            hr = mid.tile([128, NJ, CH], BF16, tag="hr")
            for m in range(NJ):
                pz = psz.tile([128, 2 * FO], F32)
                for P in range(NOPS):
                    nc.tensor.matmul(pz[:], gx[:, P, bass.ts(m, 128)],
                                     wx[:, P, 0:2 * FO], start=(P == 0), stop=False)
                for P in range(NOPS):
                    nc.tensor.matmul(pz[:], gh[:, P, bass.ts(m, 128)],
                                     wh[:, P, 0:2 * FO], start=False,
                                     stop=(P == NOPS - 1))
                nc.scalar.activation(zt[:, m, :], pz[:, 0:FO],
                                     mybir.ActivationFunctionType.Sigmoid)
                rt = spool.tile([128, FO], F32, tag="tmp")
                nc.scalar.activation(rt[:], pz[:, FO:2 * FO],
                                     mybir.ActivationFunctionType.Sigmoid)
                nc.vector.tensor_mul(hr[:, m, :], hs[:, m, b, :], rt[:])
            zs[b] = (zt, hr)

        def cand_graph(b, zs, cs):
            zt, hr = zs[b]
            gc = gcp.tile([CH, NOPS, NP], BF16, tag="gc")
            graph_ops(lambda j: hr[:, j, :], CH, gc, psg)
            cs[b] = gc

        def cand_gates(t, b, gs, zs, cs):
            gx, _ = gs[b]
            zt, _ = zs[b]
            gc = cs[b]
            for m in range(NJ):
                pc = psc.tile([128, FO], F32)
                for P in range(NOPS):
                    nc.tensor.matmul(pc[:], gx[:, P, bass.ts(m, 128)],
                                     wx[:, P, 2 * FO:], start=(P == 0), stop=False)
                for P in range(NOPS):
                    nc.tensor.matmul(pc[:], gc[:, P, bass.ts(m, 128)],
                                     wh[:, P, 2 * FO:], start=False,
                                     stop=(P == NOPS - 1))
                ht = spool.tile([128, FO], F32, tag="tmp")
                nc.scalar.activation(ht[:], pc[:],
                                     mybir.ActivationFunctionType.Tanh)
                d1 = spool.tile([128, FO], F32, tag="tmp")
                nc.vector.tensor_sub(d1[:], hs[:, m, b, :], ht[:])
                d2 = spool.tile([128, FO], F32, tag="tmp")
                nc.vector.tensor_mul(d2[:], zt[:, m, :], d1[:])
                nc.vector.tensor_add(hs[:, m, b, :], ht[:], d2[:])
                nc.vector.tensor_add(hsb[:, m, b, :], ht[:], d2[:])
            nc.sync.dma_start(y_d[:, t, :, b, :], hs[:, :, b, :])

        for t in range(T):
            gs, zs, cs = {}, {}, {}
            zr_graph(t, 0, gs)
            zr_graph(t, 1, gs)
            zr_gates(0, gs, zs)
            for b in range(BL):
                if b + 2 < BL:
                    zr_graph(t, b + 2, gs)
                cand_graph(b, zs, cs)
                if b + 1 < BL:
                    zr_gates(b + 1, gs, zs)
                cand_gates(t, b, gs, zs, cs)
    nc.compile()
    return nc


def _prep_consts(edge_index, edge_weight, Wz, bz, Wr, br, Wh, bh):
    row = edge_index[0].astype(np.int64)
    col = edge_index[1].astype(np.int64)
    w = edge_weight.astype(np.float32)
    deg_out = np.zeros(N, np.float32)
    deg_in = np.zeros(N, np.float32)
    np.add.at(deg_out, row, w)
    np.add.at(deg_in, col, w)
    norm_out = (1.0 / deg_out)[row]
    norm_in = (1.0 / deg_in)[row]  # quirk: indexed by row
    perm = np.argsort(col * N + row, kind="stable")
    A_out = np.zeros((N, N), np.float32)
    A_in = np.zeros((N, N), np.float32)
    np.add.at(A_out, (col, row), norm_out)
    np.add.at(A_in, (row[perm], col[perm]), norm_in)  # norm_in unpermuted
    I = np.eye(N, dtype=np.float32)
    A_out2 = 2.0 * (A_out @ A_out) - I
    A_in2 = 2.0 * (A_in @ A_in) - I

    amat = np.zeros((NOPS, NP, NP), np.float32)  # [P, d, s]
    for i, A in enumerate([I, A_out, A_in, A_out2, A_in2]):
        amat[i, :N, :N] = A
    # rhs layout [s%128, j, P, d]: AT[P][s, d] = A[d, s]
    amat_r = amat.transpose(2, 0, 1).reshape(NJ, 128, NOPS, NP).transpose(1, 0, 2, 3)
    amat_r = np.ascontiguousarray(amat_r)

    def terms(W):  # W: [2, 3, C, co] -> list of 5 [C, co]
        return [W[0, 0] + W[1, 0], W[0, 1], W[1, 1], W[0, 2], W[1, 2]]

    tz, tr, th = terms(Wz), terms(Wr), terms(Wh)
    wzrx = np.zeros((CX, NOPS, 2 * F_OUT), np.float32)
    wzrh = np.zeros((CH, NOPS, 2 * F_OUT), np.float32)
    whx = np.zeros((CX, NOPS, F_OUT), np.float32)
    whh = np.zeros((CH, NOPS, F_OUT), np.float32)
    for P in range(NOPS):
        wzr = np.concatenate([tz[P], tr[P]], axis=1)  # [C, 256]
        wzrx[:F_IN, P] = wzr[:F_IN]
        wzrh[:, P] = wzr[F_IN:]
        whx[:F_IN, P] = th[P][:F_IN]
        whh[:, P] = th[P][F_IN:]
    wzrx[F_IN, 0] = np.concatenate([bz, br])  # bias via ones channel, op I only
    whx[F_IN, 0] = bh
    bf = ml_dtypes.bfloat16
    return (amat_r.astype(bf), wzrx.astype(bf), wzrh.astype(bf),
            whx.astype(bf), whh.astype(bf))


def kernel(X, edge_index, edge_weight, Wz, bz, Wr, br, Wh, bh):
    X = np.asarray(X, np.float32)
    amat_r, wzrx, wzrh, whx, whh = _prep_consts(
        np.asarray(edge_index), np.asarray(edge_weight, np.float32),
        np.asarray(Wz, np.float32), np.asarray(bz, np.float32),
        np.asarray(Wr, np.float32), np.asarray(br, np.float32),
        np.asarray(Wh, np.float32), np.asarray(bh, np.float32))

    if "nc" not in _CACHE:
        _CACHE["nc"] = _build_bass()
    nc = _CACHE["nc"]

    in_maps = []
    for c in range(NC):
        Xl = X[c * BL:(c + 1) * BL]  # [BL, T, N, F_IN]
        Xp = np.zeros((BL, T, NP, CX), np.float32)
        Xp[:, :, :N, :F_IN] = Xl
        Xp[:, :, :, F_IN] = 1.0
        # -> [p, j, t, b, c]
        Xp = Xp.reshape(BL, T, NJ, 128, CX).transpose(3, 2, 1, 0, 4)
        in_maps.append({
            "xin": np.ascontiguousarray(Xp).astype(ml_dtypes.bfloat16),
            "amat": amat_r, "wzrx": wzrx, "wzrh": wzrh,
            "whx": whx, "whh": whh,
        })

    trace = bool(int(os.environ.get("KERNEL_TRACE", "0")))
    res = run_bass_kernel_spmd(nc, in_maps, core_ids=list(range(NC)), trace=trace)
    _CACHE["last_result"] = res

    out = np.empty((B, T, N, F_OUT), np.float32)
    for c in range(NC):
        y = res.results[c]["y"]  # [128, T, NJ, BL, F_OUT]
        y = y.reshape(128, T, NJ, BL, F_OUT).transpose(3, 1, 2, 0, 4)
        out[c * BL:(c + 1) * BL] = y.reshape(BL, T, NP, F_OUT)[:, :, :N, :]
    return out



# revision 24
# speedup vs baseline: 4.5555x; 1.1850x over previous
import os
import sys
from contextlib import ExitStack

import ml_dtypes
import numpy as np

sys.path.insert(0, "/opt/trn_rl_repo")

import concourse.bass as bass
from concourse import bacc
import concourse.tile as tile
from concourse import mybir
from concourse.bass_utils import run_bass_kernel_spmd

# Problem constants (hardcoded per contract)
B, T, N, F_IN, F_OUT = 64, 12, 325, 32, 128
NC = 8          # cores
BL = B // NC    # batch per core = 8
NP = 384        # padded node count for the contraction (s) dim: 3 x 128
ND = N          # destination (d) dim kept unpadded = 325
NJ = 3          # node chunks
CX = F_IN       # x channels = 32 (no ones channel; bias handled separately)
CH = F_OUT      # 128
NOPS = 5        # I, A_out, A_in, A_out2, A_in2
FO = F_OUT
F32 = mybir.dt.float32
BF16 = mybir.dt.bfloat16
# m-chunks of the destination dim (325 = 128 + 128 + 69)
MS = [(0, 128), (128, 128), (256, 69)]
GB = 3          # x-diffusion batches 3 samples at once (channel offsets 32*i;
                # SBUF base partition must be one of 0/32/64)
GROUPS = [(0, 3), (3, 3), (6, 2)]

_CACHE = {}


def _build_bass(has_bias):
    nc = bacc.Bacc(None, target_bir_lowering=False)
    x_d = nc.dram_tensor("xin", [128, NJ, T, BL, CX], BF16, kind="ExternalInput")
    a_d = nc.dram_tensor("amat", [128, NJ, NOPS, ND], BF16, kind="ExternalInput")
    # wxr: x-side weights for z|r|c, replicated at 4 partition offsets
    wxr_d = nc.dram_tensor("wxr", [96, NOPS, 3 * FO], BF16, kind="ExternalInput")
    wh_d = nc.dram_tensor("wh", [CH, NOPS, 3 * FO], BF16, kind="ExternalInput")
    if has_bias:
        brow_d = nc.dram_tensor("brow", [1, 3 * FO], BF16, kind="ExternalInput")
    y_d = nc.dram_tensor("y", [128, T, NJ, BL, FO], F32, kind="ExternalOutput")

    with tile.TileContext(nc) as tc, ExitStack() as ctx:
        const = ctx.enter_context(tc.tile_pool(name="const", bufs=1))
        state = ctx.enter_context(tc.tile_pool(name="state", bufs=1))
        ghp = ctx.enter_context(tc.tile_pool(name="ghp", bufs=3))
        gcp = ctx.enter_context(tc.tile_pool(name="gcp", bufs=2))
        gxp = ctx.enter_context(tc.tile_pool(name="gxp", bufs=2))
        actp = ctx.enter_context(tc.tile_pool(name="actp", bufs=2))
        psd = ctx.enter_context(tc.tile_pool(name="psd", bufs=2, space="PSUM"))
        psz = ctx.enter_context(tc.tile_pool(name="psz", bufs=2, space="PSUM"))
        psc = ctx.enter_context(tc.tile_pool(name="psc", bufs=2, space="PSUM"))

        xin = const.tile([128, NJ, T, BL, CX], BF16)
        amat = const.tile([128, NJ, NOPS, ND], BF16)
        wxr = const.tile([96, NOPS, 3 * FO], BF16)
        wh = const.tile([CH, NOPS, 3 * FO], BF16)
        nc.sync.dma_start(xin[:], x_d[:])
        nc.sync.dma_start(amat[:], a_d[:])
        nc.sync.dma_start(wxr[:], wxr_d[:])
        nc.sync.dma_start(wh[:], wh_d[:])
        if has_bias:
            brow = const.tile([1, 3 * FO], BF16)
            nc.sync.dma_start(brow[:], brow_d[:])
            ones = const.tile([1, 128], BF16)
            nc.gpsimd.memset(ones[:], 1.0)

        hs = state.tile([128, NJ, BL, CH], F32)   # node-major hidden state
        hsb = state.tile([128, NJ, BL, CH], BF16)  # bf16 copy for matmul lhsT
        hrs = [state.tile([128, NJ, CH], BF16, tag=f"hr{i}", name=f"hr{i}")
               for i in range(3)]
        nc.gpsimd.memset(hs[:], 0.0)
        nc.gpsimd.memset(hsb[:], 0.0)
        for h in hrs:
            nc.gpsimd.memset(h[:], 0.0)

        def diffuse(lhs_fn, cpart, gtile, copy_fn):
            # gtile[c, P, d] = sum_s lhs[s, c] * A_P[d, s]  (channel-major)
            for P in range(NOPS):
                ps = psd.tile([cpart, ND], F32)
                for j in range(NJ):
                    nc.tensor.matmul(ps[:], lhs_fn(j), amat[:, j, P, :],
                                     start=(j == 0), stop=(j == NJ - 1))
                copy_fn(gtile[:, P, :], ps[:])

        gxs, ghs, gcs, zts, rsv = {}, {}, {}, {}, {}

        def gx_make(t, g):
            b0, bw = GROUPS[g]
            gx = gxp.tile([32 * bw, NOPS, ND], BF16, tag="gx")
            diffuse(lambda j: xin[:, j, t, b0:b0 + bw, :], 32 * bw, gx,
                    nc.vector.tensor_copy)
            gxs[g] = gx

        def gh_make(b):
            gh = ghp.tile([CH, NOPS, ND], BF16, tag="gh")
            diffuse(lambda j: hsb[:, j, b, :], CH, gh, nc.scalar.copy)
            ghs[b] = gh

        def zr_gates(b):
            gx = gxs[b // GB]
            ci = 32 * (b % GB)
            gh = ghs.pop(b)
            pz = psz.tile([128, NJ, 2 * FO], F32)
            for m, (ms, mw) in enumerate(MS):
                if has_bias:
                    nc.tensor.matmul(pz[0:mw, m, :], ones[0:1, 0:mw],
                                     brow[0:1, 0:2 * FO], start=True, stop=False)
                for P in range(NOPS):
                    nc.tensor.matmul(pz[0:mw, m, :],
                                     gx[ci:ci + 32, P, ms:ms + mw],
                                     wxr[ci:ci + 32, P, 0:2 * FO],
                                     start=(not has_bias and P == 0), stop=False)
                for P in range(NOPS):
                    nc.tensor.matmul(pz[0:mw, m, :], gh[:, P, ms:ms + mw],
                                     wh[:, P, 0:2 * FO], start=False,
                                     stop=(P == NOPS - 1))
            zt = actp.tile([128, NJ, FO], F32, tag="zt")
            rt = actp.tile([128, NJ, FO], F32, tag="rt")
            nc.scalar.activation(zt[:], pz[:, :, 0:FO],
                                 mybir.ActivationFunctionType.Sigmoid)
            nc.scalar.activation(rt[:], pz[:, :, FO:2 * FO],
                                 mybir.ActivationFunctionType.Sigmoid)
            hr = hrs[b % 3]
            nc.vector.tensor_mul(hr[:, 0:2, :], hs[:, 0:2, b, :], rt[:, 0:2, :])
            nc.vector.tensor_mul(hr[0:69, 2, :], hs[0:69, 2, b, :],
                                 rt[0:69, 2, :])
            zts[b] = zt

        def cand_graph(b):
            gc = gcp.tile([CH, NOPS, ND], BF16, tag="gc")
            diffuse(lambda j: hrs[b % 3][:, j, :], CH, gc, nc.vector.tensor_copy)
            gcs[b] = gc

        def cand_gates(t, b):
            gx = gxs[b // GB]
            ci = 32 * (b % GB)
            gc = gcs.pop(b)
            zt = zts.pop(b)
            pc = psc.tile([128, NJ, FO], F32)
            for m, (ms, mw) in enumerate(MS):
                if has_bias:
                    nc.tensor.matmul(pc[0:mw, m, :], ones[0:1, 0:mw],
                                     brow[0:1, 2 * FO:], start=True, stop=False)
                for P in range(NOPS):
                    nc.tensor.matmul(pc[0:mw, m, :],
                                     gx[ci:ci + 32, P, ms:ms + mw],
                                     wxr[ci:ci + 32, P, 2 * FO:],
                                     start=(not has_bias and P == 0), stop=False)
                for P in range(NOPS):
                    nc.tensor.matmul(pc[0:mw, m, :], gc[:, P, ms:ms + mw],
                                     wh[:, P, 2 * FO:], start=False,
                                     stop=(P == NOPS - 1))
            ht = actp.tile([128, NJ, FO], F32, tag="ht")
            nc.scalar.activation(ht[:], pc[:], mybir.ActivationFunctionType.Tanh)
            d1 = actp.tile([128, NJ, FO], F32, tag="d1")
            d2 = actp.tile([128, NJ, FO], F32, tag="d2")
            # m = 0,1 full 128 partitions; m = 2 only 69 live rows (dead rows
            # must stay exactly zero so NaN garbage never reaches the PE)
            nc.vector.tensor_sub(d1[:, 0:2, :], hs[:, 0:2, b, :], ht[:, 0:2, :])
            nc.vector.tensor_sub(d1[0:69, 2, :], hs[0:69, 2, b, :],
                                 ht[0:69, 2, :])
            nc.vector.tensor_mul(d2[:, 0:2, :], zt[:, 0:2, :], d1[:, 0:2, :])
            nc.vector.tensor_mul(d2[0:69, 2, :], zt[0:69, 2, :], d1[0:69, 2, :])
            nc.vector.tensor_add(hs[:, 0:2, b, :], ht[:, 0:2, :], d2[:, 0:2, :])
            nc.vector.tensor_add(hs[0:69, 2, b, :], ht[0:69, 2, :],
                                 d2[0:69, 2, :])
            nc.vector.tensor_add(hsb[:, 0:2, b, :], ht[:, 0:2, :],
                                 d2[:, 0:2, :])
            nc.vector.tensor_add(hsb[0:69, 2, b, :], ht[0:69, 2, :],
                                 d2[0:69, 2, :])
            nc.sync.dma_start(y_d[:, t, :, b, :], hs[:, :, b, :])

        for t in range(T):
            gx_make(t, 0)
            gh_make(0)
            gh_make(1)
            zr_gates(0)
            for b in range(BL):
                if b == 1:
                    gx_make(t, 1)
                if b == 4:
                    gx_make(t, 2)
                if b + 2 < BL:
                    gh_make(b + 2)
                cand_graph(b)
                if b + 1 < BL:
                    zr_gates(b + 1)
                cand_gates(t, b)
    nc.compile()
    return nc


def _prep_consts(edge_index, edge_weight, Wz, bz, Wr, br, Wh, bh):
    row = edge_index[0].astype(np.int64)
    col = edge_index[1].astype(np.int64)
    w = edge_weight.astype(np.float32)
    deg_out = np.zeros(N, np.float32)
    deg_in = np.zeros(N, np.float32)
    np.add.at(deg_out, row, w)
    np.add.at(deg_in, col, w)
    norm_out = (1.0 / deg_out)[row]
    norm_in = (1.0 / deg_in)[row]  # quirk: indexed by row
    perm = np.argsort(col * N + row, kind="stable")
    A_out = np.zeros((N, N), np.float32)
    A_in = np.zeros((N, N), np.float32)
    np.add.at(A_out, (col, row), norm_out)
    np.add.at(A_in, (row[perm], col[perm]), norm_in)  # norm_in unpermuted
    I = np.eye(N, dtype=np.float32)
    A_out2 = 2.0 * (A_out @ A_out) - I
    A_in2 = 2.0 * (A_in @ A_in) - I

    amat = np.zeros((NOPS, NP, NP), np.float32)  # [P, d, s]
    for i, A in enumerate([I, A_out, A_in, A_out2, A_in2]):
        amat[i, :N, :N] = A
    # rhs layout [s%128, j, P, d]: AT[P][s, d] = A[d, s]; d trimmed to 325
    amat_r = amat.transpose(2, 0, 1).reshape(NJ, 128, NOPS, NP)
    amat_r = amat_r[:, :, :, :ND].transpose(1, 0, 2, 3)
    amat_r = np.ascontiguousarray(amat_r)

    def terms(W):  # W: [2, 3, C, co] -> list of 5 [C, co]
        return [W[0, 0] + W[1, 0], W[0, 1], W[1, 1], W[0, 2], W[1, 2]]

    tz, tr, th = terms(Wz), terms(Wr), terms(Wh)
    wx = np.zeros((32, NOPS, 3 * FO), np.float32)
    whf = np.zeros((CH, NOPS, 3 * FO), np.float32)
    for P in range(NOPS):
        wall = np.concatenate([tz[P], tr[P], th[P]], axis=1)  # [C, 384]
        wx[:, P] = wall[:F_IN]
        whf[:, P] = wall[F_IN:]
    wxr = np.tile(wx, (GB, 1, 1))  # [96, NOPS, 3*FO], replicated rows
    brow = np.concatenate([bz, br, bh])[None, :]  # [1, 384]
    bf = ml_dtypes.bfloat16
    return (amat_r.astype(bf), wxr.astype(bf), whf.astype(bf),
            brow.astype(bf))


def kernel(X, edge_index, edge_weight, Wz, bz, Wr, br, Wh, bh):
    X = np.asarray(X, np.float32)
    amat_r, wxr, whf, brow = _prep_consts(
        np.asarray(edge_index), np.asarray(edge_weight, np.float32),
        np.asarray(Wz, np.float32), np.asarray(bz, np.float32),
        np.asarray(Wr, np.float32), np.asarray(br, np.float32),
        np.asarray(Wh, np.float32), np.asarray(bh, np.float32))
    has_bias = bool(np.any(brow.astype(np.float32) != 0.0))

    key = ("nc", has_bias)
    if key not in _CACHE:
        _CACHE[key] = _build_bass(has_bias)
    nc = _CACHE[key]

    in_maps = []
    for c in range(NC):
        Xl = X[c * BL:(c + 1) * BL]  # [BL, T, N, F_IN]
        Xp = np.zeros((BL, T, NP, CX), np.float32)
        Xp[:, :, :N, :] = Xl
        # -> [p, j, t, b, c]
        Xp = Xp.reshape(BL, T, NJ, 128, CX).transpose(3, 2, 1, 0, 4)
        m = {
            "xin": np.ascontiguousarray(Xp).astype(ml_dtypes.bfloat16),
            "amat": amat_r, "wxr": wxr, "wh": whf,
        }
        if has_bias:
            m["brow"] = brow
        in_maps.append(m)

    trace = bool(int(os.environ.get("KERNEL_TRACE", "0")))
    res = run_bass_kernel_spmd(nc, in_maps, core_ids=list(range(NC)), trace=trace)
    _CACHE["last_result"] = res
    _CACHE["nc"] = nc  # for test.py's TimelineSim fallback

    out = np.empty((B, T, N, F_OUT), np.float32)
    for c in range(NC):
        y = res.results[c]["y"]  # [128, T, NJ, BL, F_OUT]
        y = y.reshape(128, T, NJ, BL, F_OUT).transpose(3, 1, 2, 0, 4)
        out[c * BL:(c + 1) * BL] = y.reshape(BL, T, NP, F_OUT)[:, :, :N, :]
    return out


# revision 28
# speedup vs baseline: 4.6093x; 1.0118x over previous
import os
import sys
from contextlib import ExitStack

import ml_dtypes
import numpy as np

sys.path.insert(0, "/opt/trn_rl_repo")

import concourse.bass as bass
from concourse import bacc
import concourse.tile as tile
from concourse import mybir
from concourse.bass_utils import run_bass_kernel_spmd

# Problem constants (hardcoded per contract)
B, T, N, F_IN, F_OUT = 64, 12, 325, 32, 128
NC = 8          # cores
BL = B // NC    # batch per core = 8
NP = 384        # padded node count for the contraction (s) dim: 3 x 128
ND = N          # destination (d) dim kept unpadded = 325
NJ = 3          # node chunks
CX = F_IN       # x channels = 32 (no ones channel; bias handled separately)
CH = F_OUT      # 128
NOPS = 5        # I, A_out, A_in, A_out2, A_in2
FO = F_OUT
F32 = mybir.dt.float32
BF16 = mybir.dt.bfloat16
# m-chunks of the destination dim (325 = 128 + 128 + 69)
MS = [(0, 128), (128, 128), (256, 69)]
GB = 3          # x-diffusion batches 3 samples at once (channel offsets 32*i;
                # SBUF base partition must be one of 0/32/64)
GROUPS = [(0, 3), (3, 3), (6, 2)]

_CACHE = {}


def _build_bass(has_bias):
    nc = bacc.Bacc(None, target_bir_lowering=False)
    x_d = nc.dram_tensor("xin", [128, NJ, T, BL, CX], BF16, kind="ExternalInput")
    a_d = nc.dram_tensor("amat", [128, NJ, NOPS, ND], BF16, kind="ExternalInput")
    # wxr: x-side weights for z|r|c, replicated at 4 partition offsets
    wxr_d = nc.dram_tensor("wxr", [96, NOPS, 3 * FO], BF16, kind="ExternalInput")
    wh_d = nc.dram_tensor("wh", [CH, NOPS, 3 * FO], BF16, kind="ExternalInput")
    if has_bias:
        brow_d = nc.dram_tensor("brow", [1, 3 * FO], BF16, kind="ExternalInput")
    y_d = nc.dram_tensor("y", [128, T, NJ, BL, FO], F32, kind="ExternalOutput")

    with tile.TileContext(nc) as tc, ExitStack() as ctx:
        const = ctx.enter_context(tc.tile_pool(name="const", bufs=1))
        state = ctx.enter_context(tc.tile_pool(name="state", bufs=1))
        ghp = ctx.enter_context(tc.tile_pool(name="ghp", bufs=3))
        gcp = ctx.enter_context(tc.tile_pool(name="gcp", bufs=2))
        gxp = ctx.enter_context(tc.tile_pool(name="gxp", bufs=2))
        actp = ctx.enter_context(tc.tile_pool(name="actp", bufs=2))
        psd = ctx.enter_context(tc.tile_pool(name="psd", bufs=2, space="PSUM"))
        psz = ctx.enter_context(tc.tile_pool(name="psz", bufs=2, space="PSUM"))
        psc = ctx.enter_context(tc.tile_pool(name="psc", bufs=2, space="PSUM"))

        xin = const.tile([128, NJ, T, BL, CX], BF16)
        amat = const.tile([128, NJ, NOPS, ND], BF16)
        wxr = const.tile([96, NOPS, 3 * FO], BF16)
        wh = const.tile([CH, NOPS, 3 * FO], BF16)
        nc.sync.dma_start(amat[:], a_d[:])
        nc.sync.dma_start(xin[:, :, 0, :, :], x_d[:, :, 0, :, :])
        nc.sync.dma_start(wxr[:], wxr_d[:])
        nc.sync.dma_start(wh[:], wh_d[:])
        for tt in range(1, T):
            nc.sync.dma_start(xin[:, :, tt, :, :], x_d[:, :, tt, :, :])
        if has_bias:
            brow = const.tile([1, 3 * FO], BF16)
            nc.sync.dma_start(brow[:], brow_d[:])
            ones = const.tile([1, 128], BF16)
            nc.gpsimd.memset(ones[:], 1.0)

        hs = state.tile([128, NJ, BL, CH], F32)   # node-major hidden state
        hsb = state.tile([128, NJ, BL, CH], BF16)  # bf16 copy for matmul lhsT
        hrs = [state.tile([128, NJ, CH], BF16, tag=f"hr{i}", name=f"hr{i}")
               for i in range(3)]
        nc.gpsimd.memset(hs[:], 0.0)
        nc.gpsimd.memset(hsb[:], 0.0)
        for h in hrs:
            nc.gpsimd.memset(h[:], 0.0)

        def diffuse(lhs_fn, cpart, gtile, copy_fn):
            # gtile[c, P, d] = sum_s lhs[s, c] * A_P[d, s]  (channel-major)
            for P in range(NOPS):
                ps = psd.tile([cpart, ND], F32)
                for j in range(NJ):
                    nc.tensor.matmul(ps[:], lhs_fn(j), amat[:, j, P, :],
                                     start=(j == 0), stop=(j == NJ - 1))
                copy_fn(gtile[:, P, :], ps[:])

        gxs, ghs, gcs, zts, rsv = {}, {}, {}, {}, {}

        def gx_make(t, g):
            b0, bw = GROUPS[g]
            gx = gxp.tile([32 * bw, NOPS, ND], BF16, tag="gx")
            diffuse(lambda j: xin[:, j, t, b0:b0 + bw, :], 32 * bw, gx,
                    nc.vector.tensor_copy)
            gxs[g] = gx

        def gh_make(t, b):
            gh = ghp.tile([CH, NOPS, ND], BF16, tag="gh")
            diffuse(lambda j: hsb[:, j, b, :], CH, gh, nc.scalar.copy)
            ghs[b] = gh

        def zr_gates(t, b):
            gx = gxs[b // GB]
            ci = 32 * (b % GB)
            gh = ghs.pop(b)
            pz = psz.tile([128, NJ, 2 * FO], F32)
            for m, (ms, mw) in enumerate(MS):
                if has_bias:
                    nc.tensor.matmul(pz[0:mw, m, :], ones[0:1, 0:mw],
                                     brow[0:1, 0:2 * FO], start=True, stop=False)
                for P in range(NOPS):
                    nc.tensor.matmul(pz[0:mw, m, :],
                                     gx[ci:ci + 32, P, ms:ms + mw],
                                     wxr[ci:ci + 32, P, 0:2 * FO],
                                     start=(not has_bias and P == 0), stop=False)
                for P in range(NOPS):
                    nc.tensor.matmul(pz[0:mw, m, :], gh[:, P, ms:ms + mw],
                                     wh[:, P, 0:2 * FO], start=False,
                                     stop=(P == NOPS - 1))
            zt = actp.tile([128, NJ, FO], F32, tag="zt")
            rt = actp.tile([128, NJ, FO], F32, tag="rt")
            nc.scalar.activation(zt[:], pz[:, :, 0:FO],
                                 mybir.ActivationFunctionType.Sigmoid)
            nc.scalar.activation(rt[:], pz[:, :, FO:2 * FO],
                                 mybir.ActivationFunctionType.Sigmoid)
            hr = hrs[b % 3]
            nc.vector.tensor_mul(hr[:, 0:2, :], hs[:, 0:2, b, :], rt[:, 0:2, :])
            nc.vector.tensor_mul(hr[0:69, 2, :], hs[0:69, 2, b, :],
                                 rt[0:69, 2, :])
            zts[b] = zt

        def cand_graph(b):
            gc = gcp.tile([CH, NOPS, ND], BF16, tag="gc")
            diffuse(lambda j: hrs[b % 3][:, j, :], CH, gc, nc.vector.tensor_copy)
            gcs[b] = gc

        def cand_gates(t, b):
            gx = gxs[b // GB]
            ci = 32 * (b % GB)
            gc = gcs.pop(b)
            zt = zts.pop(b)
            pc = psc.tile([128, NJ, FO], F32)
            for m, (ms, mw) in enumerate(MS):
                if has_bias:
                    nc.tensor.matmul(pc[0:mw, m, :], ones[0:1, 0:mw],
                                     brow[0:1, 2 * FO:], start=True, stop=False)
                for P in range(NOPS):
                    nc.tensor.matmul(pc[0:mw, m, :],
                                     gx[ci:ci + 32, P, ms:ms + mw],
                                     wxr[ci:ci + 32, P, 2 * FO:],
                                     start=(not has_bias and P == 0), stop=False)
                for P in range(NOPS):
                    nc.tensor.matmul(pc[0:mw, m, :], gc[:, P, ms:ms + mw],
                                     wh[:, P, 2 * FO:], start=False,
                                     stop=(P == NOPS - 1))
            ht = actp.tile([128, NJ, FO], F32, tag="ht")
            nc.scalar.activation(ht[:], pc[:], mybir.ActivationFunctionType.Tanh)
            d1 = actp.tile([128, NJ, FO], F32, tag="d1")
            d2 = actp.tile([128, NJ, FO], F32, tag="d2")
            # m = 0,1 full 128 partitions; m = 2 only 69 live rows (dead rows
            # must stay exactly zero so NaN garbage never reaches the PE)
            nc.gpsimd.tensor_sub(d1[:, 0:2, :], hs[:, 0:2, b, :], ht[:, 0:2, :])
            nc.gpsimd.tensor_sub(d1[0:69, 2, :], hs[0:69, 2, b, :],
                                 ht[0:69, 2, :])
            nc.gpsimd.tensor_mul(d2[:, 0:2, :], zt[:, 0:2, :], d1[:, 0:2, :])
            nc.gpsimd.tensor_mul(d2[0:69, 2, :], zt[0:69, 2, :],
                                 d1[0:69, 2, :])
            nc.gpsimd.tensor_add(hs[:, 0:2, b, :], ht[:, 0:2, :], d2[:, 0:2, :])
            nc.gpsimd.tensor_add(hs[0:69, 2, b, :], ht[0:69, 2, :],
                                 d2[0:69, 2, :])
            nc.vector.tensor_add(hsb[:, 0:2, b, :], ht[:, 0:2, :],
                                 d2[:, 0:2, :])
            nc.vector.tensor_add(hsb[0:69, 2, b, :], ht[0:69, 2, :],
                                 d2[0:69, 2, :])
            nc.sync.dma_start(y_d[:, t, :, b, :], hs[:, :, b, :])

        # Flat software pipeline over all (t, b): no bubbles at t boundaries.
        # Iteration k handles sample k; gh is prefetched 2 ahead, zr_gates 1
        # ahead, gx one group ahead of its first zr_gates use.
        NK = T * BL

        def gh_k(k):
            t, b = divmod(k, BL)
            gh_make(t, b)

        gx_make(0, 0)
        gh_k(0)
        gh_k(1)
        zr_gates(0, 0)
        for k in range(NK):
            t, b = divmod(k, BL)
            if (k + 1) < NK and (k + 1) % BL in (0, 3, 6):
                t1, b1 = divmod(k + 1, BL)
                gx_make(t1, b1 // GB)
            if k + 2 < NK:
                gh_k(k + 2)
            cand_graph(b)
            if k + 1 < NK:
                t1, b1 = divmod(k + 1, BL)
                zr_gates(t1, b1)
            cand_gates(t, b)
    nc.compile()
    return nc


def _prep_consts(edge_index, edge_weight, Wz, bz, Wr, br, Wh, bh):
    row = edge_index[0].astype(np.int64)
    col = edge_index[1].astype(np.int64)
    w = edge_weight.astype(np.float32)
    deg_out = np.zeros(N, np.float32)
    deg_in = np.zeros(N, np.float32)
    np.add.at(deg_out, row, w)
    np.add.at(deg_in, col, w)
    norm_out = (1.0 / deg_out)[row]
    norm_in = (1.0 / deg_in)[row]  # quirk: indexed by row
    perm = np.argsort(col * N + row, kind="stable")
    A_out = np.zeros((N, N), np.float32)
    A_in = np.zeros((N, N), np.float32)
    np.add.at(A_out, (col, row), norm_out)
    np.add.at(A_in, (row[perm], col[perm]), norm_in)  # norm_in unpermuted
    I = np.eye(N, dtype=np.float32)
    A_out2 = 2.0 * (A_out @ A_out) - I
    A_in2 = 2.0 * (A_in @ A_in) - I

    amat = np.zeros((NOPS, NP, NP), np.float32)  # [P, d, s]
    for i, A in enumerate([I, A_out, A_in, A_out2, A_in2]):
        amat[i, :N, :N] = A
    # rhs layout [s%128, j, P, d]: AT[P][s, d] = A[d, s]; d trimmed to 325
    amat_r = amat.transpose(2, 0, 1).reshape(NJ, 128, NOPS, NP)
    amat_r = amat_r[:, :, :, :ND].transpose(1, 0, 2, 3)
    amat_r = np.ascontiguousarray(amat_r)

    def terms(W):  # W: [2, 3, C, co] -> list of 5 [C, co]
        return [W[0, 0] + W[1, 0], W[0, 1], W[1, 1], W[0, 2], W[1, 2]]

    tz, tr, th = terms(Wz), terms(Wr), terms(Wh)
    wx = np.zeros((32, NOPS, 3 * FO), np.float32)
    whf = np.zeros((CH, NOPS, 3 * FO), np.float32)
    for P in range(NOPS):
        wall = np.concatenate([tz[P], tr[P], th[P]], axis=1)  # [C, 384]
        wx[:, P] = wall[:F_IN]
        whf[:, P] = wall[F_IN:]
    wxr = np.tile(wx, (GB, 1, 1))  # [96, NOPS, 3*FO], replicated rows
    brow = np.concatenate([bz, br, bh])[None, :]  # [1, 384]
    bf = ml_dtypes.bfloat16
    return (amat_r.astype(bf), wxr.astype(bf), whf.astype(bf),
            brow.astype(bf))


def kernel(X, edge_index, edge_weight, Wz, bz, Wr, br, Wh, bh):
    X = np.asarray(X, np.float32)
    amat_r, wxr, whf, brow = _prep_consts(
        np.asarray(edge_index), np.asarray(edge_weight, np.float32),
        np.asarray(Wz, np.float32), np.asarray(bz, np.float32),
        np.asarray(Wr, np.float32), np.asarray(br, np.float32),
        np.asarray(Wh, np.float32), np.asarray(bh, np.float32))
    has_bias = bool(np.any(brow.astype(np.float32) != 0.0))

    key = ("nc", has_bias)
    if key not in _CACHE:
        _CACHE[key] = _build_bass(has_bias)
    nc = _CACHE[key]

    in_maps = []
    for c in range(NC):
        Xl = X[c * BL:(c + 1) * BL]  # [BL, T, N, F_IN]
        Xp = np.zeros((BL, T, NP, CX), np.float32)
        Xp[:, :, :N, :] = Xl
        # -> [p, j, t, b, c]
        Xp = Xp.reshape(BL, T, NJ, 128, CX).transpose(3, 2, 1, 0, 4)
        m = {
            "xin": np.ascontiguousarray(Xp).astype(ml_dtypes.bfloat16),
            "amat": amat_r, "wxr": wxr, "wh": whf,
        }
        if has_bias:
            m["brow"] = brow
        in_maps.append(m)

    trace = bool(int(os.environ.get("KERNEL_TRACE", "0")))
    res = run_bass_kernel_spmd(nc, in_maps, core_ids=list(range(NC)), trace=trace)
    _CACHE["last_result"] = res
    _CACHE["nc"] = nc  # for test.py's TimelineSim fallback

    out = np.empty((B, T, N, F_OUT), np.float32)
    for c in range(NC):
        y = res.results[c]["y"]  # [128, T, NJ, BL, F_OUT]
        y = y.reshape(128, T, NJ, BL, F_OUT).transpose(3, 1, 2, 0, 4)
        out[c * BL:(c + 1) * BL] = y.reshape(BL, T, NP, F_OUT)[:, :, :N, :]
    return out


# revision 35
# speedup vs baseline: 4.8503x; 1.0523x over previous
import os
import sys
from contextlib import ExitStack

import ml_dtypes
import numpy as np

sys.path.insert(0, "/opt/trn_rl_repo")

import concourse.bass as bass
from concourse import bacc
import concourse.tile as tile
from concourse import mybir
from concourse.bass_utils import run_bass_kernel_spmd

# Problem constants (hardcoded per contract)
B, T, N, F_IN, F_OUT = 64, 12, 325, 32, 128
NC = 8          # cores
BL = B // NC    # batch per core = 8
NP = 384        # padded node count for the contraction (s) dim: 3 x 128
ND = N          # destination (d) dim kept unpadded = 325
NJ = 3          # node chunks
CX = F_IN       # x channels = 32 (no ones channel; bias handled separately)
CH = F_OUT      # 128
NOPS = 5        # I, A_out, A_in, A_out2, A_in2
FO = F_OUT
F32 = mybir.dt.float32
BF16 = mybir.dt.bfloat16
# m-chunks of the destination dim (325 = 128 + 128 + 69)
MS = [(0, 128), (128, 128), (256, 69)]
GB = 3          # x-diffusion batches 3 samples at once (channel offsets 32*i;
                # SBUF base partition must be one of 0/32/64)
GROUPS = [(0, 3), (3, 3), (6, 2)]

_CACHE = {}


def _build_bass(has_bias):
    nc = bacc.Bacc(None, target_bir_lowering=False)
    x_d = nc.dram_tensor("xin", [128, NJ, T, BL, CX], BF16, kind="ExternalInput")
    a_d = nc.dram_tensor("amat", [128, NJ, NOPS, ND], BF16, kind="ExternalInput")
    # wxr: x-side weights for z|r|c, replicated at 4 partition offsets
    wxr_d = nc.dram_tensor("wxr", [96, NOPS, 3 * FO], BF16, kind="ExternalInput")
    wh_d = nc.dram_tensor("wh", [CH, NOPS, 3 * FO], BF16, kind="ExternalInput")
    if has_bias:
        brow_d = nc.dram_tensor("brow", [1, 3 * FO], BF16, kind="ExternalInput")
    i_d = nc.dram_tensor("ident", [128, 128], BF16, kind="ExternalInput")
    y_d = nc.dram_tensor("y", [128, T, NJ, BL, FO], F32, kind="ExternalOutput")

    with tile.TileContext(nc) as tc, ExitStack() as ctx:
        const = ctx.enter_context(tc.tile_pool(name="const", bufs=1))
        state = ctx.enter_context(tc.tile_pool(name="state", bufs=1))
        ghp = ctx.enter_context(tc.tile_pool(name="ghp", bufs=3))
        gcp = ctx.enter_context(tc.tile_pool(name="gcp", bufs=2))
        gxp = ctx.enter_context(tc.tile_pool(name="gxp", bufs=2))
        actp = ctx.enter_context(tc.tile_pool(name="actp", bufs=2))
        psd = ctx.enter_context(tc.tile_pool(name="psd", bufs=3, space="PSUM"))
        psz = ctx.enter_context(tc.tile_pool(name="psz", bufs=2, space="PSUM"))
        psc = ctx.enter_context(tc.tile_pool(name="psc", bufs=1, space="PSUM"))

        xin = const.tile([128, NJ, T, BL, CX], BF16)
        amat = const.tile([128, NJ, NOPS, ND], BF16)
        wxr = const.tile([96, NOPS, 3 * FO], BF16)
        wh = const.tile([CH, NOPS, 3 * FO], BF16)
        for P in range(NOPS):
            nc.sync.dma_start(amat[:, :, P, :], a_d[:, :, P, :])
        nc.sync.dma_start(xin[:, :, 0, :, :], x_d[:, :, 0, :, :])
        nc.sync.dma_start(wxr[:], wxr_d[:])
        nc.sync.dma_start(wh[:], wh_d[:])
        for tt in range(1, T):
            nc.sync.dma_start(xin[:, :, tt, :, :], x_d[:, :, tt, :, :])
        ident = const.tile([128, 128], BF16)
        nc.sync.dma_start(ident[:], i_d[:])
        if has_bias:
            brow = const.tile([1, 3 * FO], BF16)
            nc.sync.dma_start(brow[:], brow_d[:])
            ones = const.tile([1, 128], BF16)
            nc.gpsimd.memset(ones[:], 1.0)

        hs = state.tile([128, NJ, BL, CH], F32)   # node-major hidden state
        hsb = state.tile([128, NJ, BL, CH], BF16)  # bf16 copy for matmul lhsT
        hrs = [state.tile([128, NJ, CH], BF16, tag=f"hr{i}", name=f"hr{i}")
               for i in range(3)]
        nc.gpsimd.memset(hs[:], 0.0)
        nc.gpsimd.memset(hsb[:], 0.0)
        for h in hrs:
            nc.gpsimd.memset(h[:], 0.0)

        def diffuse(lhs_fn, cpart, gtile, copy_fn):
            # gtile[c, P, d] = sum_s lhs[s, c] * A_P[d, s]  (channel-major)
            # P = 0 is the identity op: done as 3 PE transposes instead of
            # streaming the dense 325-wide identity block.
            ps = psd.tile([cpart, ND], F32)
            psb = ps[:].bitcast(BF16)  # [cpart, 2*ND] bf16 view
            for j in range(NJ):
                nc.tensor.transpose(psb[:, 128 * j:128 * (j + 1)], lhs_fn(j),
                                    ident[:])
            copy_fn(gtile[:, 0, :], psb[:, 0:ND])
            for P in range(1, NOPS):
                ps = psd.tile([cpart, ND], F32)
                for j in range(NJ):
                    nc.tensor.matmul(ps[:], lhs_fn(j), amat[:, j, P, :],
                                     start=(j == 0), stop=(j == NJ - 1))
                copy_fn(gtile[:, P, :], ps[:])

        gxs, ghs, gcs, zts, rsv = {}, {}, {}, {}, {}

        def gx_make(t, g):
            b0, bw = GROUPS[g]
            gx = gxp.tile([32 * bw, NOPS, ND], BF16, tag="gx")
            diffuse(lambda j: xin[:, j, t, b0:b0 + bw, :], 32 * bw, gx,
                    nc.vector.tensor_copy)
            gxs[g] = gx

        def gh_make(t, b):
            gh = ghp.tile([CH, NOPS, ND], BF16, tag="gh")
            diffuse(lambda j: hsb[:, j, b, :], CH, gh, nc.scalar.copy)
            ghs[b] = gh

        def zr_gates(t, b):
            gx = gxs[b // GB]
            ci = 32 * (b % GB)
            gh = ghs.pop(b)
            pz = psz.tile([128, NJ, 2 * FO], F32)
            for m, (ms, mw) in enumerate(MS):
                if has_bias:
                    nc.tensor.matmul(pz[0:mw, m, :], ones[0:1, 0:mw],
                                     brow[0:1, 0:2 * FO], start=True, stop=False)
                for P in range(NOPS):
                    nc.tensor.matmul(pz[0:mw, m, :],
                                     gx[ci:ci + 32, P, ms:ms + mw],
                                     wxr[ci:ci + 32, P, 0:2 * FO],
                                     start=(not has_bias and P == 0), stop=False)
                for P in range(NOPS):
                    nc.tensor.matmul(pz[0:mw, m, :], gh[:, P, ms:ms + mw],
                                     wh[:, P, 0:2 * FO], start=False,
                                     stop=(P == NOPS - 1))
            zrt = actp.tile([128, NJ, 2 * FO], F32, tag="zrt")
            nc.scalar.activation(zrt[:], pz[:],
                                 mybir.ActivationFunctionType.Sigmoid)
            hr = hrs[b % 3]
            nc.vector.tensor_mul(hr[:, 0:2, :], hs[:, 0:2, b, :],
                                 zrt[:, 0:2, FO:2 * FO])
            nc.vector.tensor_mul(hr[0:69, 2, :], hs[0:69, 2, b, :],
                                 zrt[0:69, 2, FO:2 * FO])
            zts[b] = zrt

        def cand_graph(b):
            gc = gcp.tile([CH, NOPS, ND], BF16, tag="gc")
            diffuse(lambda j: hrs[b % 3][:, j, :], CH, gc, nc.vector.tensor_copy)
            gcs[b] = gc

        def cand_gates(t, b):
            gx = gxs[b // GB]
            ci = 32 * (b % GB)
            gc = gcs.pop(b)
            zt = zts.pop(b)
            pc = psc.tile([128, NJ, FO], F32)
            for m, (ms, mw) in enumerate(MS):
                if has_bias:
                    nc.tensor.matmul(pc[0:mw, m, :], ones[0:1, 0:mw],
                                     brow[0:1, 2 * FO:], start=True, stop=False)
                for P in range(NOPS):
                    nc.tensor.matmul(pc[0:mw, m, :],
                                     gx[ci:ci + 32, P, ms:ms + mw],
                                     wxr[ci:ci + 32, P, 2 * FO:],
                                     start=(not has_bias and P == 0), stop=False)
                for P in range(NOPS):
                    nc.tensor.matmul(pc[0:mw, m, :], gc[:, P, ms:ms + mw],
                                     wh[:, P, 2 * FO:], start=False,
                                     stop=(P == NOPS - 1))
            ht = actp.tile([128, NJ, FO], F32, tag="ht")
            nc.scalar.activation(ht[:], pc[:], mybir.ActivationFunctionType.Tanh)
            d1 = actp.tile([128, NJ, FO], F32, tag="d1")
            d2 = actp.tile([128, NJ, FO], F32, tag="d2")
            # m = 0,1 full 128 partitions; m = 2 only 69 live rows (dead rows
            # must stay exactly zero so NaN garbage never reaches the PE)
            nc.gpsimd.tensor_sub(d1[:, 0:2, :], hs[:, 0:2, b, :], ht[:, 0:2, :])
            nc.gpsimd.tensor_sub(d1[0:69, 2, :], hs[0:69, 2, b, :],
                                 ht[0:69, 2, :])
            nc.gpsimd.tensor_mul(d2[:, 0:2, :], zt[:, 0:2, 0:FO],
                                 d1[:, 0:2, :])
            nc.gpsimd.tensor_mul(d2[0:69, 2, :], zt[0:69, 2, 0:FO],
                                 d1[0:69, 2, :])
            nc.gpsimd.tensor_add(hs[:, 0:2, b, :], ht[:, 0:2, :], d2[:, 0:2, :])
            nc.gpsimd.tensor_add(hs[0:69, 2, b, :], ht[0:69, 2, :],
                                 d2[0:69, 2, :])
            nc.vector.tensor_add(hsb[:, 0:2, b, :], ht[:, 0:2, :],
                                 d2[:, 0:2, :])
            nc.vector.tensor_add(hsb[0:69, 2, b, :], ht[0:69, 2, :],
                                 d2[0:69, 2, :])
            nc.sync.dma_start(y_d[:, t, :, b, :], hs[:, :, b, :])

        # Flat software pipeline over all (t, b): no bubbles at t boundaries.
        # Iteration k handles sample k; gh is prefetched 2 ahead, zr_gates 1
        # ahead, gx one group ahead of its first zr_gates use.
        NK = T * BL

        def gh_k(k):
            t, b = divmod(k, BL)
            gh_make(t, b)

        gx_make(0, 0)
        gh_k(0)
        gh_k(1)
        zr_gates(0, 0)
        for k in range(NK):
            t, b = divmod(k, BL)
            if (k + 1) < NK and (k + 1) % BL in (0, 3, 6):
                t1, b1 = divmod(k + 1, BL)
                gx_make(t1, b1 // GB)
            if k + 2 < NK:
                gh_k(k + 2)
            cand_graph(b)
            if k + 1 < NK:
                t1, b1 = divmod(k + 1, BL)
                zr_gates(t1, b1)
            cand_gates(t, b)
    nc.compile()
    return nc


def _prep_consts(edge_index, edge_weight, Wz, bz, Wr, br, Wh, bh):
    row = edge_index[0].astype(np.int64)
    col = edge_index[1].astype(np.int64)
    w = edge_weight.astype(np.float32)
    deg_out = np.zeros(N, np.float32)
    deg_in = np.zeros(N, np.float32)
    np.add.at(deg_out, row, w)
    np.add.at(deg_in, col, w)
    norm_out = (1.0 / deg_out)[row]
    norm_in = (1.0 / deg_in)[row]  # quirk: indexed by row
    perm = np.argsort(col * N + row, kind="stable")
    A_out = np.zeros((N, N), np.float32)
    A_in = np.zeros((N, N), np.float32)
    np.add.at(A_out, (col, row), norm_out)
    np.add.at(A_in, (row[perm], col[perm]), norm_in)  # norm_in unpermuted
    I = np.eye(N, dtype=np.float32)
    A_out2 = 2.0 * (A_out @ A_out) - I
    A_in2 = 2.0 * (A_in @ A_in) - I

    amat = np.zeros((NOPS, NP, NP), np.float32)  # [P, d, s]
    for i, A in enumerate([I, A_out, A_in, A_out2, A_in2]):
        amat[i, :N, :N] = A
    # rhs layout [s%128, j, P, d]: AT[P][s, d] = A[d, s]; d trimmed to 325
    amat_r = amat.transpose(2, 0, 1).reshape(NJ, 128, NOPS, NP)
    amat_r = amat_r[:, :, :, :ND].transpose(1, 0, 2, 3)
    amat_r = np.ascontiguousarray(amat_r)

    def terms(W):  # W: [2, 3, C, co] -> list of 5 [C, co]
        return [W[0, 0] + W[1, 0], W[0, 1], W[1, 1], W[0, 2], W[1, 2]]

    tz, tr, th = terms(Wz), terms(Wr), terms(Wh)
    wx = np.zeros((32, NOPS, 3 * FO), np.float32)
    whf = np.zeros((CH, NOPS, 3 * FO), np.float32)
    for P in range(NOPS):
        wall = np.concatenate([tz[P], tr[P], th[P]], axis=1)  # [C, 384]
        wx[:, P] = wall[:F_IN]
        whf[:, P] = wall[F_IN:]
    wxr = np.tile(wx, (GB, 1, 1))  # [96, NOPS, 3*FO], replicated rows
    brow = np.concatenate([bz, br, bh])[None, :]  # [1, 384]
    bf = ml_dtypes.bfloat16
    return (amat_r.astype(bf), wxr.astype(bf), whf.astype(bf),
            brow.astype(bf))


def kernel(X, edge_index, edge_weight, Wz, bz, Wr, br, Wh, bh):
    X = np.asarray(X, np.float32)
    amat_r, wxr, whf, brow = _prep_consts(
        np.asarray(edge_index), np.asarray(edge_weight, np.float32),
        np.asarray(Wz, np.float32), np.asarray(bz, np.float32),
        np.asarray(Wr, np.float32), np.asarray(br, np.float32),
        np.asarray(Wh, np.float32), np.asarray(bh, np.float32))
    has_bias = bool(np.any(brow.astype(np.float32) != 0.0))

    key = ("nc", has_bias)
    if key not in _CACHE:
        _CACHE[key] = _build_bass(has_bias)
    nc = _CACHE[key]

    in_maps = []
    for c in range(NC):
        Xl = X[c * BL:(c + 1) * BL]  # [BL, T, N, F_IN]
        Xp = np.zeros((BL, T, NP, CX), np.float32)
        Xp[:, :, :N, :] = Xl
        # -> [p, j, t, b, c]
        Xp = Xp.reshape(BL, T, NJ, 128, CX).transpose(3, 2, 1, 0, 4)
        m = {
            "xin": np.ascontiguousarray(Xp).astype(ml_dtypes.bfloat16),
            "amat": amat_r, "wxr": wxr, "wh": whf,
            "ident": np.eye(128, dtype=np.float32).astype(ml_dtypes.bfloat16),
        }
        if has_bias:
            m["brow"] = brow
        in_maps.append(m)

    trace = bool(int(os.environ.get("KERNEL_TRACE", "0")))
    res = run_bass_kernel_spmd(nc, in_maps, core_ids=list(range(NC)), trace=trace)
    _CACHE["last_result"] = res
    _CACHE["nc"] = nc  # for test.py's TimelineSim fallback

    out = np.empty((B, T, N, F_OUT), np.float32)
    for c in range(NC):
        y = res.results[c]["y"]  # [128, T, NJ, BL, F_OUT]
        y = y.reshape(128, T, NJ, BL, F_OUT).transpose(3, 1, 2, 0, 4)
        out[c * BL:(c + 1) * BL] = y.reshape(BL, T, NP, F_OUT)[:, :, :N, :]
    return out


# revision 52
# speedup vs baseline: 5.2930x; 1.0913x over previous
import os
import sys
from contextlib import ExitStack

import ml_dtypes
import numpy as np

sys.path.insert(0, "/opt/trn_rl_repo")

import concourse.bass as bass
from concourse import bacc
import concourse.tile as tile
from concourse import mybir
from concourse.bass_utils import run_bass_kernel_spmd

# Problem constants (hardcoded per contract)
B, T, N, F_IN, F_OUT = 64, 12, 325, 32, 128
NC = 8          # cores
BL = B // NC    # batch per core = 8
NP = 384        # padded node count for the contraction (s) dim: 3 x 128
ND = N          # destination (d) dim kept unpadded = 325
NJ = 3          # node chunks
CX = F_IN       # x channels = 32 (no ones channel; bias handled separately)
CH = F_OUT      # 128
NOPS = 5        # I, A_out, A_in, A_out2, A_in2
FO = F_OUT
F32 = mybir.dt.float32
BF16 = mybir.dt.bfloat16
# m-chunks of the destination dim (325 = 128 + 128 + 69)
MS = [(0, 128), (128, 128), (256, 69)]
SJ = [128, 128, 69]  # live source rows per node chunk (325 = 128+128+69)

_CACHE = {}


def _build_bass(has_bias):
    nc = bacc.Bacc(None, target_bir_lowering=False)
    x_d = nc.dram_tensor("xin", [128, NJ, T, BL, CX], BF16, kind="ExternalInput")
    a_d = nc.dram_tensor("amat", [128, NJ, NOPS, ND], BF16, kind="ExternalInput")
    # x-side weights with (P, c) folded onto partitions: wf1 holds P0-2 at
    # offsets 0/32/64, wf2 holds P3-4 at offsets 0/32. cols are z|r|c.
    wf1_d = nc.dram_tensor("wf1", [96, 3 * FO], BF16, kind="ExternalInput")
    wf2_d = nc.dram_tensor("wf2", [64, 3 * FO], BF16, kind="ExternalInput")
    wh_d = nc.dram_tensor("wh", [CH, NOPS, 3 * FO], BF16, kind="ExternalInput")
    if has_bias:
        brow_d = nc.dram_tensor("brow", [1, 3 * FO], BF16, kind="ExternalInput")
    i_d = nc.dram_tensor("ident", [128, 128], BF16, kind="ExternalInput")
    y_d = nc.dram_tensor("y", [128, T, NJ, BL, FO], F32, kind="ExternalOutput")

    with tile.TileContext(nc) as tc, ExitStack() as ctx:
        const = ctx.enter_context(tc.tile_pool(name="const", bufs=1))
        state = ctx.enter_context(tc.tile_pool(name="state", bufs=1))
        ghp = ctx.enter_context(tc.tile_pool(name="ghp", bufs=3))
        gcp = ctx.enter_context(tc.tile_pool(name="gcp", bufs=2))
        gxp = ctx.enter_context(tc.tile_pool(name="gxp", bufs=3))
        actp = ctx.enter_context(tc.tile_pool(name="actp", bufs=2))
        psd = ctx.enter_context(tc.tile_pool(name="psd", bufs=3, space="PSUM"))
        psz = ctx.enter_context(tc.tile_pool(name="psz", bufs=2, space="PSUM"))
        psc = ctx.enter_context(tc.tile_pool(name="psc", bufs=1, space="PSUM"))

        xin = const.tile([128, NJ, T, BL, CX], BF16)
        amat = const.tile([128, NJ, NOPS - 1, ND], BF16)  # ops 1..4 (0 = I)
        wf1 = const.tile([96, 3 * FO], BF16)
        wf2 = const.tile([64, 3 * FO], BF16)
        wh = const.tile([CH, NOPS, 3 * FO], BF16)
        ident = const.tile([128, 128], BF16)
        nc.sync.dma_start(ident[:], i_d[:])
        nc.sync.dma_start(xin[:, :, 0, :, :], x_d[:, :, 0, :, :])
        for P in range(NOPS - 1):
            for j in range(NJ):
                nc.sync.dma_start(amat[:, j, P, :], a_d[:, j, P + 1, :])
        nc.sync.dma_start(wf1[:], wf1_d[:])
        nc.sync.dma_start(wf2[:], wf2_d[:])
        nc.sync.dma_start(wh[:], wh_d[:])
        for tt in range(1, T):
            nc.sync.dma_start(xin[:, :, tt, :, :], x_d[:, :, tt, :, :])
        if has_bias:
            brow = const.tile([1, 3 * FO], BF16)
            nc.sync.dma_start(brow[:], brow_d[:])
            ones = const.tile([1, 128], BF16)
            nc.gpsimd.memset(ones[:], 1.0)

        hs = state.tile([128, NJ, BL, CH], F32)   # node-major hidden state
        hsb = state.tile([128, NJ, BL, CH], BF16)  # bf16 copy for matmul lhsT
        hrs = [state.tile([128, NJ, CH], BF16, tag=f"hr{i}", name=f"hr{i}")
               for i in range(3)]
        nc.gpsimd.memset(hs[:], 0.0)
        nc.gpsimd.memset(hsb[:], 0.0)
        for h in hrs:
            nc.gpsimd.memset(h[:], 0.0)

        def diffuse(lhs_fn, lhsT_fn, cpart, gtile, copy_fn):
            # gtile[c, P, d] = sum_s lhs[s, c] * A_P[d, s]  (channel-major)
            # P = 0 is the identity op: done as 3 PE transposes instead of
            # streaming the dense 325-wide identity block.
            ps = psd.tile([cpart, ND], F32, tag="ps", name="ps")
            psb = ps[:].bitcast(BF16)  # [cpart, 2*ND] bf16 view
            for j in range(NJ):
                nc.tensor.transpose(psb[:, 128 * j:128 * j + SJ[j]],
                                    lhsT_fn(j), ident[0:SJ[j], 0:SJ[j]])
            copy_fn(gtile[:, 0, :], psb[:, 0:ND])
            for P in range(1, NOPS):
                ps = psd.tile([cpart, ND], F32, tag="ps", name="ps")
                for j in range(NJ):
                    nc.tensor.matmul(ps[:], lhs_fn(j), amat[:, j, P - 1, :],
                                     start=(j == 0), stop=(j == NJ - 1))
                copy_fn(gtile[:, P, :], ps[:])

        gxs, ghs, gcs, zts = {}, {}, {}, {}

        def gx_make(t, b, k):
            # x diffusion with (P, c=32) folded onto partitions:
            #   gx1 [96, d] holds P0..2 at offsets 0/32/64, gx2 [64, d] P3..4
            ps1 = psd.tile([96, ND], F32, tag="ps", name="ps")
            ps2 = psd.tile([64, ND], F32, tag="ps", name="ps")
            ps1b = ps1[0:32, :].bitcast(BF16)
            for j in range(NJ):
                nc.tensor.transpose(ps1b[:, 128 * j:128 * j + SJ[j]],
                                    xin[0:SJ[j], j, t, b, :],
                                    ident[0:SJ[j], 0:SJ[j]])
            for pi, (pst, off) in enumerate([(ps1, 32), (ps1, 64),
                                             (ps2, 0), (ps2, 32)]):
                for j in range(NJ):
                    nc.tensor.matmul(pst[off:off + 32, :],
                                     xin[:, j, t, b, :], amat[:, j, pi, :],
                                     start=(j == 0), stop=(j == NJ - 1))
            gx1 = gxp.tile([96, ND], BF16, tag="gx1")
            gx2 = gxp.tile([64, ND], BF16, tag="gx2")
            nc.vector.tensor_copy(gx1[0:32, :], ps1b[:, 0:ND])
            nc.vector.tensor_copy(gx1[32:64, :], ps1[32:64, :])
            nc.scalar.copy(gx1[64:96, :], ps1[64:96, :])
            nc.vector.tensor_copy(gx2[:], ps2[:])
            gxs[k] = (gx1, gx2)

        def gh_make(t, b):
            gh = ghp.tile([CH, NOPS, ND], BF16, tag="gh")
            diffuse(lambda j: hsb[:, j, b, :],
                    lambda j: hsb[0:SJ[j], j, b, :], CH, gh, nc.scalar.copy)
            ghs[b] = gh

        def zr_gates(t, b, k):
            gx1, gx2 = gxs[k]
            gh = ghs.pop(b)
            pz = psz.tile([128, NJ, 2 * FO], F32)
            for m, (ms, mw) in enumerate(MS):
                if has_bias:
                    nc.tensor.matmul(pz[0:mw, m, :], ones[0:1, 0:mw],
                                     brow[0:1, 0:2 * FO], start=True, stop=False)
                nc.tensor.matmul(pz[0:mw, m, :], gx1[:, ms:ms + mw],
                                 wf1[:, 0:2 * FO],
                                 start=(not has_bias), stop=False)
                nc.tensor.matmul(pz[0:mw, m, :], gx2[:, ms:ms + mw],
                                 wf2[:, 0:2 * FO], start=False, stop=False)
                for P in range(NOPS):
                    nc.tensor.matmul(pz[0:mw, m, :], gh[:, P, ms:ms + mw],
                                     wh[:, P, 0:2 * FO], start=False,
                                     stop=(P == NOPS - 1))
            zrt = actp.tile([128, NJ, 2 * FO], F32, tag="zrt")
            nc.scalar.activation(zrt[:], pz[:],
                                 mybir.ActivationFunctionType.Sigmoid)
            hr = hrs[b % 3]
            nc.vector.tensor_mul(hr[:, 0:2, :], hs[:, 0:2, b, :],
                                 zrt[:, 0:2, FO:2 * FO])
            nc.vector.tensor_mul(hr[0:69, 2, :], hs[0:69, 2, b, :],
                                 zrt[0:69, 2, FO:2 * FO])
            zts[b] = zrt

        def cand_graph(b):
            gc = gcp.tile([CH, NOPS, ND], BF16, tag="gc")
            diffuse(lambda j: hrs[b % 3][:, j, :],
                    lambda j: hrs[b % 3][0:SJ[j], j, :], CH, gc,
                    nc.vector.tensor_copy)
            gcs[b] = gc

        def cand_gates(t, b, k):
            gx1, gx2 = gxs.pop(k)
            gc = gcs.pop(b)
            zt = zts.pop(b)
            pc = psc.tile([128, NJ, FO], F32)
            for m, (ms, mw) in enumerate(MS):
                if has_bias:
                    nc.tensor.matmul(pc[0:mw, m, :], ones[0:1, 0:mw],
                                     brow[0:1, 2 * FO:], start=True, stop=False)
                nc.tensor.matmul(pc[0:mw, m, :], gx1[:, ms:ms + mw],
                                 wf1[:, 2 * FO:],
                                 start=(not has_bias), stop=False)
                nc.tensor.matmul(pc[0:mw, m, :], gx2[:, ms:ms + mw],
                                 wf2[:, 2 * FO:], start=False, stop=False)
                for P in range(NOPS):
                    nc.tensor.matmul(pc[0:mw, m, :], gc[:, P, ms:ms + mw],
                                     wh[:, P, 2 * FO:], start=False,
                                     stop=(P == NOPS - 1))
            ht = actp.tile([128, NJ, FO], F32, tag="ht")
            nc.scalar.activation(ht[:], pc[:], mybir.ActivationFunctionType.Tanh)
            d1 = actp.tile([128, NJ, FO], F32, tag="d1")
            d2 = actp.tile([128, NJ, FO], F32, tag="d2")
            # m = 0,1 full 128 partitions; m = 2 only 69 live rows (dead rows
            # must stay exactly zero so NaN garbage never reaches the PE)
            nc.gpsimd.tensor_sub(d1[:, 0:2, :], hs[:, 0:2, b, :], ht[:, 0:2, :])
            nc.gpsimd.tensor_sub(d1[0:69, 2, :], hs[0:69, 2, b, :],
                                 ht[0:69, 2, :])
            nc.gpsimd.tensor_mul(d2[:, 0:2, :], zt[:, 0:2, 0:FO],
                                 d1[:, 0:2, :])
            nc.gpsimd.tensor_mul(d2[0:69, 2, :], zt[0:69, 2, 0:FO],
                                 d1[0:69, 2, :])
            nc.gpsimd.tensor_add(hs[:, 0:2, b, :], ht[:, 0:2, :], d2[:, 0:2, :])
            nc.gpsimd.tensor_add(hs[0:69, 2, b, :], ht[0:69, 2, :],
                                 d2[0:69, 2, :])
            nc.vector.tensor_add(hsb[:, 0:2, b, :], ht[:, 0:2, :],
                                 d2[:, 0:2, :])
            nc.vector.tensor_add(hsb[0:69, 2, b, :], ht[0:69, 2, :],
                                 d2[0:69, 2, :])
            nc.sync.dma_start(y_d[:, t, :, b, :], hs[:, :, b, :])

        # Flat software pipeline over all (t, b): no bubbles at t boundaries.
        # Iteration k handles sample k; gh is prefetched 2 ahead, zr_gates 1
        # ahead, gx one group ahead of its first zr_gates use.
        NK = T * BL

        def gh_k(k):
            t, b = divmod(k, BL)
            gh_make(t, b)

        gx_make(0, 0, 0)
        gh_k(0)
        gx_make(0, 1, 1)
        gh_k(1)
        zr_gates(0, 0, 0)
        for k in range(NK):
            t, b = divmod(k, BL)
            if k + 2 < NK:
                gh_k(k + 2)
                t2, b2 = divmod(k + 2, BL)
                gx_make(t2, b2, k + 2)
            cand_graph(b)
            if k + 1 < NK:
                t1, b1 = divmod(k + 1, BL)
                zr_gates(t1, b1, k + 1)
            cand_gates(t, b, k)
    nc.compile()
    return nc


def _prep_consts(edge_index, edge_weight, Wz, bz, Wr, br, Wh, bh):
    row = edge_index[0].astype(np.int64)
    col = edge_index[1].astype(np.int64)
    w = edge_weight.astype(np.float32)
    deg_out = np.zeros(N, np.float32)
    deg_in = np.zeros(N, np.float32)
    np.add.at(deg_out, row, w)
    np.add.at(deg_in, col, w)
    norm_out = (1.0 / deg_out)[row]
    norm_in = (1.0 / deg_in)[row]  # quirk: indexed by row
    perm = np.argsort(col * N + row, kind="stable")
    A_out = np.zeros((N, N), np.float32)
    A_in = np.zeros((N, N), np.float32)
    np.add.at(A_out, (col, row), norm_out)
    np.add.at(A_in, (row[perm], col[perm]), norm_in)  # norm_in unpermuted
    I = np.eye(N, dtype=np.float32)
    A_out2 = 2.0 * (A_out @ A_out) - I
    A_in2 = 2.0 * (A_in @ A_in) - I

    amat = np.zeros((NOPS, NP, NP), np.float32)  # [P, d, s]
    for i, A in enumerate([I, A_out, A_in, A_out2, A_in2]):
        amat[i, :N, :N] = A
    # rhs layout [s%128, j, P, d]: AT[P][s, d] = A[d, s]; d trimmed to 325
    amat_r = amat.transpose(2, 0, 1).reshape(NJ, 128, NOPS, NP)
    amat_r = amat_r[:, :, :, :ND].transpose(1, 0, 2, 3)
    amat_r = np.ascontiguousarray(amat_r)

    def terms(W):  # W: [2, 3, C, co] -> list of 5 [C, co]
        return [W[0, 0] + W[1, 0], W[0, 1], W[1, 1], W[0, 2], W[1, 2]]

    tz, tr, th = terms(Wz), terms(Wr), terms(Wh)
    wx = np.zeros((32, NOPS, 3 * FO), np.float32)
    whf = np.zeros((CH, NOPS, 3 * FO), np.float32)
    for P in range(NOPS):
        wall = np.concatenate([tz[P], tr[P], th[P]], axis=1)  # [C, 384]
        wx[:, P] = wall[:F_IN]
        whf[:, P] = wall[F_IN:]
    # fold (P, c) onto rows: wfold[32*P + c] = wx[c, P]
    wfold = wx.transpose(1, 0, 2).reshape(NOPS * 32, 3 * FO)
    brow = np.concatenate([bz, br, bh])[None, :]  # [1, 384]
    bf = ml_dtypes.bfloat16
    return (amat_r.astype(bf), wfold[:96].astype(bf), wfold[96:].astype(bf),
            whf.astype(bf), brow.astype(bf))


def kernel(X, edge_index, edge_weight, Wz, bz, Wr, br, Wh, bh):
    X = np.asarray(X, np.float32)
    amat_r, wf1, wf2, whf, brow = _prep_consts(
        np.asarray(edge_index), np.asarray(edge_weight, np.float32),
        np.asarray(Wz, np.float32), np.asarray(bz, np.float32),
        np.asarray(Wr, np.float32), np.asarray(br, np.float32),
        np.asarray(Wh, np.float32), np.asarray(bh, np.float32))
    has_bias = bool(np.any(brow.astype(np.float32) != 0.0))

    key = ("nc", has_bias)
    if key not in _CACHE:
        _CACHE[key] = _build_bass(has_bias)
    nc = _CACHE[key]

    in_maps = []
    for c in range(NC):
        Xl = X[c * BL:(c + 1) * BL]  # [BL, T, N, F_IN]
        Xp = np.zeros((BL, T, NP, CX), np.float32)
        Xp[:, :, :N, :] = Xl
        # -> [p, j, t, b, c]
        Xp = Xp.reshape(BL, T, NJ, 128, CX).transpose(3, 2, 1, 0, 4)
        m = {
            "xin": np.ascontiguousarray(Xp).astype(ml_dtypes.bfloat16),
            "amat": amat_r, "wf1": wf1, "wf2": wf2, "wh": whf,
            "ident": np.eye(128, dtype=np.float32).astype(ml_dtypes.bfloat16),
        }
        if has_bias:
            m["brow"] = brow
        in_maps.append(m)

    trace = bool(int(os.environ.get("KERNEL_TRACE", "0")))
    res = run_bass_kernel_spmd(nc, in_maps, core_ids=list(range(NC)), trace=trace)
    _CACHE["last_result"] = res
    _CACHE["nc"] = nc  # for test.py's TimelineSim fallback

    out = np.empty((B, T, N, F_OUT), np.float32)
    for c in range(NC):
        y = res.results[c]["y"]  # [128, T, NJ, BL, F_OUT]
        y = y.reshape(128, T, NJ, BL, F_OUT).transpose(3, 1, 2, 0, 4)
        out[c * BL:(c + 1) * BL] = y.reshape(BL, T, NP, F_OUT)[:, :, :N, :]
    return out
